# revision 33
# baseline (speedup 1.0000x reference)
"""Trainium2 Bass kernel for the Gudi UpProj block (fp8 DoubleRow version).

Reference computation (per image, NCHW):
    xu  = zero_stuff_2x(x)                    # [B,1024,32,32], nonzero only at even (h,w)
    c1  = conv5x5(xu, w1, pad=2);  out1 = relu(BN(c1))
    c2  = conv3x3(out1, w2, pad=1)
    csc = conv5x5(xu, wsc, pad=2)
    out = relu(BN(c2) + BN(csc))              # BN: training-mode batch stats over (N,H,W)

Strategy:
  * Data-parallel over batch: 16 images -> 2 per NeuronCore (8 cores).
  * Zero-stuffing: 5x5 conv on the zero-stuffed 32x32 grid decomposes into 4
    parity phases, each a small conv on the 16x16 grid -> 4x FLOP reduction.
  * All convs in fp8e4 (e4m3) with MatmulPerfMode.DoubleRow: each matmul
    contracts 2 k-blocks (K=256) at 0.5 PE cycles/row.  Accuracy is recovered
    with a hi+lo split of BOTH operands at a shared power-of-2 scale
    (x = xh + xl, w = wh + wl, all e4m3 at the same scale), computing
    xh*wh + xl*wh + xh*wl in one PSUM accumulation group.  The dropped
    xl*wl term is ~0.1% relative -> ~0.2-0.3% on the final output.
    Per-tensor scales are free: BN is scale-invariant (eps rescaled).
  * BN batch stats cross-core exchange via small AllGather collectives
    (cheaper than AllReduce) + local sum; stats for c1 and csc are overlapped
    with compute, only the c2-stats gather sits on the tail.
"""

import numpy as np
import ml_dtypes

NCORES = 8
B = 16
B_LOC = B // NCORES          # 2 images per core
CIN, COUT = 1024, 512
NCI, NCO = CIN // 128, COUT // 128   # 8, 4 partition tiles
NPAIR = NCI // 2              # 4 cin-tile pairs (DoubleRow k-groups)
H = 16                        # input spatial
OH = 32                       # output spatial
CNT = float(B * OH * OH)      # BN element count per channel = 16384
PHASES = [(0, 0), (0, 1), (1, 0), (1, 1)]

SX = 32.0                     # x quantization scale
SW = 1024.0                   # conv5 weight scale (w1, wsc)
SA = 32.0                     # out1 activation scale
SW2 = 1024.0                  # conv2 weight scale
ALPHA = SX * SW               # = SA * SW2: scale of every conv output
EPS_S = 1e-5 * ALPHA * ALPHA  # BN eps in the scaled-variance domain

F8NP = ml_dtypes.float8_e4m3

_CACHE = {}


def _taps(p):
    """Taps of a parity phase along one dim: list of (input shift, 5-tap kernel idx)."""
    if p == 0:
        return [(-1, 0), (0, 2), (1, 4)]
    return [(0, 1), (1, 3)]


def _w5_groups():
    """Weight-block groups for the phase-decomposed 5x5 conv in consumption
    order: one group per (phase, cin-pair, kernel-row) holding len(kws) blocks
    of [128 k, 2 member, COUT]."""
    groups = []
    for (p, q) in PHASES:
        for j in range(NPAIR):
            for (ah, kh) in _taps(p):
                groups.append((p, q, j, ah, kh, _taps(q)))
    return groups


def _phase_view(ap2048, p, q):
    """[128, 2048] tile viewed as [128, b, i, j] at output positions (2i+p, 2j+q)."""
    v = ap2048.rearrange("c (b i p2 j q2) -> c b i p2 j q2", b=2, i=16, p2=2, j=16, q2=2)
    return v[:, :, :, p, :, q]


def _build_nc():
    import concourse.mybir as mybir
    import concourse.tile as tile
    from concourse import bacc

    f32 = mybir.dt.float32
    f8 = mybir.dt.float8e4
    ALU = mybir.AluOpType
    AFT = mybir.ActivationFunctionType
    DR = mybir.MatmulPerfMode.DoubleRow

    nc = bacc.Bacc("TRN2", target_bir_lowering=False, debug=False)

    # ---- kernel I/O ----
    xh_d = nc.dram_tensor("xh", [NPAIR, 128, 2, B_LOC, 18, 18], f8, kind="ExternalInput").ap()
    xl_d = nc.dram_tensor("xl", [NPAIR, 128, 2, B_LOC, 18, 18], f8, kind="ExternalInput").ap()
    w1h_d = nc.dram_tensor("w1h", [100, 128, 2, COUT], f8, kind="ExternalInput").ap()
    w1l_d = nc.dram_tensor("w1l", [100, 128, 2, COUT], f8, kind="ExternalInput").ap()
    wsch_d = nc.dram_tensor("wsch", [100, 128, 2, COUT], f8, kind="ExternalInput").ap()
    wscl_d = nc.dram_tensor("wscl", [100, 128, 2, COUT], f8, kind="ExternalInput").ap()
    w2h_d = nc.dram_tensor("w2h", [2, 128, 9, 2, COUT], f8, kind="ExternalInput").ap()
    w2l_d = nc.dram_tensor("w2l", [2, 128, 9, 2, COUT], f8, kind="ExternalInput").ap()
    gb_d = nc.dram_tensor("gb", [128, 6, 4], f32, kind="ExternalInput").ap()
    zp_d = nc.dram_tensor("zp", [2, B_LOC, 34, 34], f8, kind="ExternalInput").ap()
    out_d = nc.dram_tensor("out", [B_LOC, COUT, OH, OH], f32, kind="ExternalOutput").ap()

    with tile.TileContext(nc) as tc:
        # collective buffers (internal DRAM)
        _frees = []

        def dram(shape, shared=False, name=""):
            t, _f = tc.tile(shape, f32, space="DRAM",
                            addr_space="Shared" if shared else None, name=name)
            _frees.append(_f)
            return t

        # stats layouts are [s(sum,sq), c(128), co] so DMAs keep co contiguous
        ag1_in = dram([2, 128, 4], name="ag1_in")
        ag1_out = dram([NCORES, 2, 128, 4], shared=True, name="ag1_out")
        agsc_in = dram([2, 128, 4], name="agsc_in")
        agsc_out = dram([NCORES, 2, 128, 4], shared=True, name="agsc_out")
        ag2a_in = dram([2, 128, 2], name="ag2a_in")
        ag2a_out = dram([NCORES, 2, 128, 2], shared=True, name="ag2a_out")
        ag2b_in = dram([2, 128, 1], name="ag2b_in")
        ag2b_out = dram([NCORES, 2, 128, 1], shared=True, name="ag2b_out")
        ag2c_in = dram([2, 128, 1], name="ag2c_in")
        ag2c_out = dram([NCORES, 2, 128, 1], shared=True, name="ag2c_out")

        with tc.tile_pool(name="xp", bufs=1) as xp_pool, \
             tc.tile_pool(name="acts", bufs=1) as acts, \
             tc.tile_pool(name="op1", bufs=1) as op1_pool, \
             tc.tile_pool(name="w2p", bufs=1) as w2p, \
             tc.tile_pool(name="wts", bufs=8) as wts, \
             tc.tile_pool(name="tsc", bufs=2) as tsc_pool, \
             tc.tile_pool(name="scr", bufs=1) as scr_pool, \
             tc.tile_pool(name="small", bufs=1) as small, \
             tc.tile_pool(name="ps", bufs=8, space="PSUM") as ps:

            # ---- persistent SBUF tensors ----
            XH = [xp_pool.tile([128, 2, B_LOC, 18, 18], f8, name=f"xh{j}", tag=f"xh{j}")
                  for j in range(NPAIR)]
            XL = [xp_pool.tile([128, 2, B_LOC, 18, 18], f8, name=f"xl{j}", tag=f"xl{j}")
                  for j in range(NPAIR)]
            C1 = [acts.tile([128, 2048], f32, name=f"c1_{i}", tag=f"c1_{i}") for i in range(NCO)]
            CSC = [acts.tile([128, 2048], f32, name=f"csc_{i}", tag=f"csc_{i}") for i in range(NCO)]
            C2 = C1           # conv1 results are dead once BN1 is applied
            FIN = CSC         # csc results are dead once the final fuse read them
            O1H = [op1_pool.tile([128, 2, B_LOC, 34, 34], f8, name=f"o1h{j}", tag=f"o1h{j}")
                   for j in range(2)]
            O1L = [op1_pool.tile([128, 2, B_LOC, 34, 34], f8, name=f"o1l{j}", tag=f"o1l{j}")
                   for j in range(2)]
            W2H = [w2p.tile([128, 9, 2, COUT], f8, name=f"w2h{cp}", tag=f"w2h{cp}")
                   for cp in range(2)]
            W2L = [w2p.tile([128, 9, 2, COUT], f8, name=f"w2l{cp}", tag=f"w2l{cp}")
                   for cp in range(2)]

            # stat columns: sums/sumsqs per (tensor, co, phase-or-quarter)
            sums1 = small.tile([128, 16], f32, name="sums1")
            sq1 = small.tile([128, 16], f32, name="sq1")
            sums2 = small.tile([128, 16], f32, name="sums2")
            sq2 = small.tile([128, 16], f32, name="sq2")
            sumssc = small.tile([128, 16], f32, name="sumssc")
            sqsc = small.tile([128, 16], f32, name="sqsc")
            pack1 = small.tile([128, 2, 4], f32, name="pack1")
            packsc = small.tile([128, 2, 4], f32, name="packsc")
            pack2a = small.tile([128, 2, 2], f32, name="pack2a")
            pack2b = small.tile([128, 2, 1], f32, name="pack2b")
            pack2c = small.tile([128, 2, 1], f32, name="pack2c")
            st1g = small.tile([128, NCORES, 2, 4], f32, name="st1g")
            stscg = small.tile([128, NCORES, 2, 4], f32, name="stscg")
            st2ga = small.tile([128, NCORES, 2, 2], f32, name="st2ga")
            st2gb = small.tile([128, NCORES, 2], f32, name="st2gb")
            st2gc = small.tile([128, NCORES, 2], f32, name="st2gc")
            st1 = small.tile([128, 2, 4], f32, name="st1")
            st2 = small.tile([128, 2, 2, 4], f32, name="st2")   # [c, grp(c2,sc), (sum,sq), co]
            gbv = small.tile([128, 6, 4], f32, name="gbv")      # rows: SA*g1, SA*b1, g2, b2, gsc, bsc
            scale1 = small.tile([128, 4], f32, name="scale1")
            shift1 = small.tile([128, 4], f32, name="shift1")
            tmpa = small.tile([128, 4], f32, name="tmpa")
            tmpb = small.tile([128, 4], f32, name="tmpb")
            epsc = small.tile([128, 1], f32, name="epsc")
            # two independent sets of BN2/BNsc coefficient tiles (pass a: cos
            # 0-1 fused early; pass b: cos 2-3 fused on the tail)
            coefs = {}
            for ph in ("a", "b", "c"):
                coefs[ph] = {
                    "stm": small.tile([128, 2, 2, 4], f32, name=f"stm_{ph}"),
                    "m2": small.tile([128, 2, 4], f32, name=f"m2_{ph}"),
                    "var": small.tile([128, 2, 4], f32, name=f"var_{ph}"),
                    "inv": small.tile([128, 2, 4], f32, name=f"inv_{ph}"),
                    "scaleb": small.tile([128, 2, 4], f32, name=f"scaleb_{ph}"),
                    "shiftb2": small.tile([128, 2, 4], f32, name=f"shiftb2_{ph}"),
                    "shiftB": small.tile([128, 4], f32, name=f"shiftB_{ph}"),
                    "rmix": small.tile([128, 4], f32, name=f"rmix_{ph}"),
                    "tmp": small.tile([128, 4], f32, name=f"tmp_{ph}"),
                }

            # ---- input DMAs (x first: the PE's first dependency) ----
            x_emitted = [True] + [False] * (NPAIR - 1)

            def emit_x(j):
                if not x_emitted[j]:
                    x_emitted[j] = True
                    nc.sync.dma_start(XH[j][:], xh_d[j])
                    nc.sync.dma_start(XL[j][:], xl_d[j])

            nc.sync.dma_start(XH[0][:], xh_d[0])
            nc.vector.memset(epsc[:], EPS_S)
            nc.vector.memset(st2[:], 1.0)

            def gsum(dst, gtile, w):
                """dst[c, 2, W] = sum over cores of gtile[c, 8, 2, W]."""
                nc.vector.tensor_reduce(
                    dst, gtile[:].rearrange("c g s co -> c s co g"),
                    axis=mybir.AxisListType.X, op=ALU.add)

            # ---- helper: one 5x5-phase-decomposed conv (conv1 / convsc) ----
            def conv5(wh_d, wl_d, dst, sums, sqs, wtag, prefetch_x=False):
                gofs = 0
                for iph, (p, q) in enumerate(PHASES):
                    pps = [ps.tile([128, B_LOC, 256], f32, name=f"{wtag}ps{iph}_{co}", tag="psb")
                           for co in range(NCO)]
                    kws = _taps(q)
                    L = len(kws)
                    total = NPAIR * len(_taps(p)) * L * 3 * B_LOC
                    cnt = [0] * NCO
                    for j in range(NPAIR):
                        if prefetch_x and iph == 0 and j + 1 < NPAIR:
                            emit_x(j + 1)
                        for (ah, kh) in _taps(p):
                            wh = wts.tile([128, 3, 2, COUT], f8, name=f"{wtag}wh", tag="w5")
                            wl = wts.tile([128, 3, 2, COUT], f8, name=f"{wtag}wl", tag="w5")
                            if gofs == 0 and prefetch_x:
                                # split the very first group so the first
                                # matmul's weights land as early as possible;
                                # xl0 is only needed 8 matmuls in, so it rides
                                # behind the first hi-path blocks
                                nc.sync.dma_start(
                                    wh[:, 0:1], wh_d[0:1].rearrange("l k m o -> k l m o"))
                                nc.sync.dma_start(
                                    wl[:, 0:1], wl_d[0:1].rearrange("l k m o -> k l m o"))
                                nc.sync.dma_start(XL[0][:], xl_d[0])
                                nc.sync.dma_start(
                                    wh[:, 1:L], wh_d[1:L].rearrange("l k m o -> k l m o"))
                                nc.sync.dma_start(
                                    wl[:, 1:L], wl_d[1:L].rearrange("l k m o -> k l m o"))
                            else:
                                nc.sync.dma_start(
                                    wh[:, :L], wh_d[gofs:gofs + L].rearrange("l k m o -> k l m o"))
                                nc.sync.dma_start(
                                    wl[:, :L], wl_d[gofs:gofs + L].rearrange("l k m o -> k l m o"))
                            gofs += L
                            for kwi, (aw, kw) in enumerate(kws):
                                for (xt, wt) in ((XH, wh), (XL, wh), (XH, wl)):
                                    for b in range(B_LOC):
                                        rhs = xt[j][:, :, b, 1 + ah:17 + ah, 1 + aw:17 + aw]
                                        for co in range(NCO):
                                            nc.tensor.matmul(
                                                pps[co][:, b],
                                                wt[:, kwi, :, co * 128:(co + 1) * 128],
                                                rhs,
                                                start=(cnt[co] == 0),
                                                stop=(cnt[co] == total - 1),
                                                perf_mode=DR)
                                            cnt[co] += 1
                    for co in range(NCO):
                        icol = co * 4 + iph
                        nc.vector.tensor_scalar(
                            dst[co][:, iph * 512:(iph + 1) * 512],
                            pps[co][:].rearrange("c b n -> c (b n)"),
                            0.0, 0.0, op0=ALU.add, op1=ALU.add,
                            accum_out=sums[:, icol:icol + 1])
                        scr = scr_pool.tile([128, 512], f32, name=f"{wtag}scr", tag="scr")
                        nc.scalar.activation(
                            scr[:], pps[co][:].rearrange("c b n -> c (b n)"), AFT.Square,
                            accum_out=sqs[:, icol:icol + 1])

            # ================= conv1 =================
            conv5(w1h_d, w1l_d, C1, sums1, sq1, "c1", prefetch_x=True)

            # aux DMAs (needed later; emitted after the conv1 weight stream so
            # they don't delay it on the DMA engines)
            nc.sync.dma_start(gbv[:], gb_d)
            for cp in range(2):
                nc.sync.dma_start(W2H[cp][:], w2h_d[cp])
                nc.sync.dma_start(W2L[cp][:], w2l_d[cp])
                nc.sync.dma_start(O1H[cp][:], zp_d.unsqueeze(0).partition_broadcast(128))
                nc.sync.dma_start(O1L[cp][:], zp_d.unsqueeze(0).partition_broadcast(128))

            # ---- c1 stats -> AllGather #1 (overlaps with convsc compute) ----
            nc.vector.tensor_reduce(
                pack1[:, 0, :], sums1[:].rearrange("c (co ph) -> c co ph", ph=4),
                axis=mybir.AxisListType.X, op=ALU.add)
            nc.vector.tensor_reduce(
                pack1[:, 1, :], sq1[:].rearrange("c (co ph) -> c co ph", ph=4),
                axis=mybir.AxisListType.X, op=ALU.add)
            nc.sync.dma_start(ag1_in[:].rearrange("s c co -> c s co"), pack1[:])
            nc.gpsimd.collective_compute(
                "AllGather", ALU.bypass,
                replica_groups=[list(range(NCORES))],
                ins=[ag1_in.opt()], outs=[ag1_out.opt()])

            # ================= convsc (independent of BN1) =================
            conv5(wsch_d, wscl_d, CSC, sumssc, sqsc, "sc")

            # ---- csc stats -> AllGather (overlaps with conv2) ----
            nc.vector.tensor_reduce(
                packsc[:, 0, :], sumssc[:].rearrange("c (co ph) -> c co ph", ph=4),
                axis=mybir.AxisListType.X, op=ALU.add)
            nc.vector.tensor_reduce(
                packsc[:, 1, :], sqsc[:].rearrange("c (co ph) -> c co ph", ph=4),
                axis=mybir.AxisListType.X, op=ALU.add)
            nc.sync.dma_start(agsc_in[:].rearrange("s c co -> c s co"), packsc[:])
            nc.gpsimd.collective_compute(
                "AllGather", ALU.bypass,
                replica_groups=[list(range(NCORES))],
                ins=[agsc_in.opt()], outs=[agsc_out.opt()])

            # ---- BN1 scale/shift from global stats (incl. SA requant scale) ----
            nc.sync.dma_start(st1g[:], ag1_out[:].rearrange("g s c co -> c g s co"))
            gsum(st1[:], st1g, 4)
            nc.vector.tensor_scalar_mul(st1[:], st1[:], 1.0 / CNT)
            m1 = st1[:, 0, :]
            nc.vector.tensor_tensor(tmpa[:], m1, m1, op=ALU.mult)
            nc.vector.tensor_tensor(tmpb[:], st1[:, 1, :], tmpa[:], op=ALU.subtract)
            nc.scalar.activation(tmpb[:], tmpb[:], AFT.Sqrt, bias=epsc[:])
            nc.vector.reciprocal(tmpa[:], tmpb[:])
            nc.vector.tensor_tensor(scale1[:], gbv[:, 0, :], tmpa[:], op=ALU.mult)
            nc.vector.tensor_tensor(tmpa[:], m1, scale1[:], op=ALU.mult)
            nc.vector.tensor_tensor(shift1[:], gbv[:, 1, :], tmpa[:], op=ALU.subtract)

            # ---- BN1 apply + ReLU + fp8 hi/lo requant -> padded conv2 input ----
            for co in range(NCO):
                cp, m = co // 2, co % 2
                for iph, (p, q) in enumerate(PHASES):
                    src = C1[co][:, iph * 512:(iph + 1) * 512]
                    T = tsc_pool.tile([128, 512], f32, name="bn1T", tag="bn1T")
                    nc.scalar.activation(T[:], src, AFT.Relu,
                                         bias=shift1[:, co:co + 1], scale=scale1[:, co:co + 1])
                    Tv = T[:].rearrange("c (b h w) -> c b h w", b=B_LOC, h=16)
                    hdst = O1H[cp][:, m, :, 1:33, 1:33] \
                        .rearrange("c b (i p2) (jj q2) -> c b i p2 jj q2", p2=2, q2=2)[:, :, :, p, :, q]
                    ldst = O1L[cp][:, m, :, 1:33, 1:33] \
                        .rearrange("c b (i p2) (jj q2) -> c b i p2 jj q2", p2=2, q2=2)[:, :, :, p, :, q]
                    nc.scalar.activation(hdst, Tv, AFT.Copy)
                    nc.vector.tensor_tensor(ldst, Tv, hdst, op=ALU.subtract)

            # ================= conv2 (3x3, pad 1, on O1 fp8 hi/lo) =================
            # quarter-outer loop: each quarter's PSUM group completes early so
            # its copy-out overlaps the next quarter's matmuls.  For the LAST
            # co the stats come from a cheap tensor_reduce instead of the
            # (strided, slow) copy's accumulator, so the tail collective isn't
            # gated on the copy.
            def conv2_co(co, split_stats=False):
                total = 2 * 9 * 3 * B_LOC
                for qq in range(4):
                    pp = ps.tile([128, B_LOC, 256], f32, name=f"c2ps{co}_{qq}", tag="psb")
                    cnt = 0
                    for cp in range(2):
                        for dh in (-1, 0, 1):
                            for dw in (-1, 0, 1):
                                t = (dh + 1) * 3 + (dw + 1)
                                for (xt, wt) in ((O1H, W2H), (O1L, W2H), (O1H, W2L)):
                                    for b in range(B_LOC):
                                        rhs = xt[cp][:, :, b,
                                                     1 + 8 * qq + dh:9 + 8 * qq + dh,
                                                     1 + dw:33 + dw]
                                        nc.tensor.matmul(
                                            pp[:, b],
                                            wt[cp][:, t, :, co * 128:(co + 1) * 128],
                                            rhs,
                                            start=(cnt == 0),
                                            stop=(cnt == total - 1),
                                            perf_mode=DR)
                                        cnt += 1
                    icol = co * 4 + qq
                    dstv = C2[co][:].rearrange("c (b h w) -> c b h w", b=B_LOC, h=32)[:, :, 8 * qq:8 * qq + 8, :]
                    scr = scr_pool.tile([128, 512], f32, name="c2scr", tag="scr")
                    if split_stats:
                        pf = pp[:].rearrange("c b n -> c (b n)")
                        nc.vector.tensor_reduce(
                            sums2[:, icol:icol + 1], pf,
                            axis=mybir.AxisListType.X, op=ALU.add)
                        nc.scalar.activation(
                            scr[:], pf, AFT.Square,
                            accum_out=sq2[:, icol:icol + 1])
                        nc.vector.tensor_scalar(
                            dstv, pp[:].rearrange("c b (h w) -> c b h w", h=8),
                            0.0, 0.0, op0=ALU.add, op1=ALU.add)
                    else:
                        nc.vector.tensor_scalar(
                            dstv, pp[:].rearrange("c b (h w) -> c b h w", h=8),
                            0.0, 0.0, op0=ALU.add, op1=ALU.add,
                            accum_out=sums2[:, icol:icol + 1])
                        nc.scalar.activation(
                            scr[:], pp[:].rearrange("c b n -> c (b n)"), AFT.Square,
                            accum_out=sq2[:, icol:icol + 1])

            def pack_c2(pk, ag_in, ag_out, lo, hi):
                nc.vector.tensor_reduce(
                    pk[:, 0, :], sums2[:, lo * 4:hi * 4].rearrange("c (co x) -> c co x", x=4),
                    axis=mybir.AxisListType.X, op=ALU.add)
                nc.vector.tensor_reduce(
                    pk[:, 1, :], sq2[:, lo * 4:hi * 4].rearrange("c (co x) -> c co x", x=4),
                    axis=mybir.AxisListType.X, op=ALU.add)
                nc.sync.dma_start(ag_in[:].rearrange("s c co -> c s co"), pk[:])
                nc.gpsimd.collective_compute(
                    "AllGather", ALU.bypass,
                    replica_groups=[list(range(NCORES))],
                    ins=[ag_in.opt()], outs=[ag_out.opt()])

            def bn2_coefs(cf):
                """BN2/BNsc scale+shift from st2 into the given coef tile set.
                final = relu(s2*c2 + t2 + ssc*csc + tsc)
                      = relu( s2 * (c2 + (ssc/s2)*csc) + (t2 + tsc) )"""
                stm = cf["stm"]
                nc.vector.tensor_scalar_mul(stm[:], st2[:], 1.0 / CNT)
                means = stm[:, :, 0, :]    # [c, 2, 4]
                e2s = stm[:, :, 1, :]
                gpair = gbv[:, 2:, :].rearrange("c (g s) co -> c g s co", s=2)
                nc.vector.tensor_tensor(cf["m2"][:], means, means, op=ALU.mult)
                nc.vector.tensor_tensor(cf["var"][:], e2s, cf["m2"][:], op=ALU.subtract)
                nc.scalar.activation(cf["var"][:], cf["var"][:], AFT.Sqrt, bias=epsc[:])
                nc.vector.reciprocal(cf["inv"][:], cf["var"][:])
                nc.vector.tensor_tensor(cf["scaleb"][:], gpair[:, :, 0, :], cf["inv"][:], op=ALU.mult)
                nc.vector.tensor_tensor(cf["m2"][:], means, cf["scaleb"][:], op=ALU.mult)
                nc.vector.tensor_tensor(cf["shiftb2"][:], gpair[:, :, 1, :], cf["m2"][:], op=ALU.subtract)
                nc.vector.tensor_tensor(cf["shiftB"][:], cf["shiftb2"][:, 0, :],
                                        cf["shiftb2"][:, 1, :], op=ALU.add)
                nc.vector.reciprocal(cf["tmp"][:], cf["scaleb"][:, 0, :])
                nc.vector.tensor_tensor(cf["rmix"][:], cf["scaleb"][:, 1, :], cf["tmp"][:], op=ALU.mult)

            def fuse_co(co, cf):
                """c2 += rmix*csc ; out = relu(scale2*c2 + shiftB) ; DMA out."""
                for iph, (p, q) in enumerate(PHASES):
                    nc.vector.scalar_tensor_tensor(
                        _phase_view(C2[co][:], p, q),
                        CSC[co][:, iph * 512:(iph + 1) * 512]
                        .rearrange("c (b h w) -> c b h w", b=B_LOC, h=16),
                        cf["rmix"][:, co:co + 1],
                        _phase_view(C2[co][:], p, q),
                        op0=ALU.mult, op1=ALU.add)
                fin = FIN[co]
                for b in range(B_LOC):
                    nc.scalar.activation(fin[:, b * 1024:(b + 1) * 1024],
                                         C2[co][:, b * 1024:(b + 1) * 1024], AFT.Relu,
                                         bias=cf["shiftB"][:, co:co + 1],
                                         scale=cf["scaleb"][:, 0, co:co + 1])
                    nc.scalar.dma_start(
                        out_d[b, co * 128:(co + 1) * 128].rearrange("c h w -> c (h w)"),
                        fin[:, b * 1024:(b + 1) * 1024])

            conv2_co(0)
            conv2_co(1)
            # stats for cos 0-1 -> AllGather (hidden under cos 2-3 conv2);
            # their BN coefs + fuse + output DMA also overlap cos 2-3 compute.
            pack_c2(pack2a, ag2a_in, ag2a_out, 0, 2)
            conv2_co(2)
            pack_c2(pack2b, ag2b_in, ag2b_out, 2, 3)
            conv2_co(3, split_stats=True)
            pack_c2(pack2c, ag2c_in, ag2c_out, 3, 4)

            # gather-backs + local sums (csc stats gathered during conv2)
            nc.sync.dma_start(stscg[:], agsc_out[:].rearrange("g s c co -> c g s co"))
            gsum(st2[:, 1], stscg, 4)
            nc.sync.dma_start(st2ga[:], ag2a_out[:].rearrange("g s c co -> c g s co"))
            gsum(st2[:, 0, :, 0:2], st2ga, 2)
            bn2_coefs(coefs["a"])
            fuse_co(0, coefs["a"])
            fuse_co(1, coefs["a"])

            nc.sync.dma_start(st2gb[:], ag2b_out[:].rearrange("g s c co -> c g (s co)"))
            nc.vector.tensor_reduce(st2[:, 0, :, 2], st2gb[:].rearrange("c g s -> c s g"), axis=mybir.AxisListType.X, op=ALU.add)
            bn2_coefs(coefs["b"])
            fuse_co(2, coefs["b"])

            nc.sync.dma_start(st2gc[:], ag2c_out[:].rearrange("g s c co -> c g (s co)"))
            nc.vector.tensor_reduce(st2[:, 0, :, 3], st2gc[:].rearrange("c g s -> c s g"), axis=mybir.AxisListType.X, op=ALU.add)
            bn2_coefs(coefs["c"])
            fuse_co(3, coefs["c"])

            for _f in _frees:
                _f()

    nc.compile()
    return nc


def _get_nc():
    if "nc" not in _CACHE:
        _CACHE["nc"] = _build_nc()
    return _CACHE["nc"]


def _q8pair(a):
    """Split fp32 array into e4m3 hi + lo at unit scale (caller pre-scales)."""
    hi = a.astype(F8NP)
    lo = (a - hi.astype(np.float32)).astype(F8NP)
    return hi, lo


def _regroup_w5(wt_full):
    """[5,5,CIN,COUT] (pre-scaled fp32) -> [100, 128, 2, COUT] blocks in
    kernel consumption order."""
    blocks = np.empty((100, 128, 2, COUT), dtype=np.float32)
    g = 0
    for (p, q, j, ah, kh, kws) in _w5_groups():
        for (aw, kw) in kws:
            blk = wt_full[kh, kw, j * 256:(j + 1) * 256, :]          # [256, COUT]
            blocks[g] = blk.reshape(2, 128, COUT).transpose(1, 0, 2)  # [k, m, o]
            g += 1
    assert g == 100
    return blocks


def _prep_core_x(xpad_s):
    """Scaled padded x for one core [B_LOC, CIN, 18, 18] ->
    [NPAIR, 128, 2, B_LOC, 18, 18] (pair, k, member, img, h, w)."""
    a = xpad_s.reshape(B_LOC, NPAIR, 2, 128, 18, 18)
    return np.ascontiguousarray(a.transpose(1, 3, 2, 0, 4, 5))


def _prep_inputs(x, w1, w2, wsc, g1, b1, g2, b2, gsc, bsc):
    xpad = np.zeros((B, CIN, 18, 18), dtype=np.float32)
    xpad[:, :, 1:17, 1:17] = x
    xpad *= SX

    w1g = _regroup_w5(np.ascontiguousarray(w1.transpose(2, 3, 1, 0)) * SW)
    wscg = _regroup_w5(np.ascontiguousarray(wsc.transpose(2, 3, 1, 0)) * SW)
    w1h, w1l = _q8pair(w1g)
    wsch, wscl = _q8pair(wscg)

    # w2: [COUT, COUT, 3, 3] -> [cp, k, t, m, o]
    wt2 = np.ascontiguousarray(w2.transpose(2, 3, 1, 0)).reshape(9, COUT, COUT) * SW2
    w2g = np.ascontiguousarray(
        wt2.reshape(9, 2, 2, 128, COUT).transpose(1, 3, 0, 2, 4))  # [cp, k, t, m, o]
    w2h, w2l = _q8pair(w2g)

    gb = np.stack([g1 * SA, b1 * SA, g2, b2, gsc, bsc]).astype(np.float32)   # [6, 512]
    gbt = np.ascontiguousarray(gb.reshape(6, 4, 128).transpose(2, 0, 1))  # [128, 6, 4]
    return xpad, (w1h, w1l), (wsch, wscl), (w2h, w2l), gbt


def kernel(x, w1, g1, b1, w2, g2, b2, wsc, gsc, bsc, _trace=False, **_kw):
    from concourse.bass_utils import run_bass_kernel_spmd

    x = np.asarray(x, dtype=np.float32)
    xpad, (w1h, w1l), (wsch, wscl), (w2h, w2l), gbt = _prep_inputs(
        np.asarray(x), np.asarray(w1), np.asarray(w2), np.asarray(wsc),
        np.asarray(g1), np.asarray(b1), np.asarray(g2), np.asarray(b2),
        np.asarray(gsc), np.asarray(bsc))

    nc = _get_nc()
    zp = np.zeros((2, B_LOC, 34, 34), dtype=F8NP)
    in_maps = []
    for core in range(NCORES):
        xs = _prep_core_x(xpad[core * B_LOC:(core + 1) * B_LOC])
        xh, xl = _q8pair(xs)
        in_maps.append({
            "xh": xh, "xl": xl,
            "w1h": w1h, "w1l": w1l, "wsch": wsch, "wscl": wscl,
            "w2h": w2h, "w2l": w2l, "gb": gbt, "zp": zp,
        })
    try:
        res = run_bass_kernel_spmd(nc, in_maps, list(range(NCORES)), trace=_trace)
    except ModuleNotFoundError:
        # NTFF profile hook unavailable under this axon client
        res = run_bass_kernel_spmd(nc, in_maps, list(range(NCORES)), trace=False)
    out = np.concatenate([res.results[i]["out"] for i in range(NCORES)], axis=0)
    if _trace:
        _CACHE["last_result"] = res
    return out


# revision 34
# speedup vs baseline: 1.0017x; 1.0017x over previous
"""Trainium2 Bass kernel for the Gudi UpProj block (fp8 DoubleRow version).

Reference computation (per image, NCHW):
    xu  = zero_stuff_2x(x)                    # [B,1024,32,32], nonzero only at even (h,w)
    c1  = conv5x5(xu, w1, pad=2);  out1 = relu(BN(c1))
    c2  = conv3x3(out1, w2, pad=1)
    csc = conv5x5(xu, wsc, pad=2)
    out = relu(BN(c2) + BN(csc))              # BN: training-mode batch stats over (N,H,W)

Strategy:
  * Data-parallel over batch: 16 images -> 2 per NeuronCore (8 cores).
  * Zero-stuffing: 5x5 conv on the zero-stuffed 32x32 grid decomposes into 4
    parity phases, each a small conv on the 16x16 grid -> 4x FLOP reduction.
  * All convs in fp8e4 (e4m3) with MatmulPerfMode.DoubleRow: each matmul
    contracts 2 k-blocks (K=256) at 0.5 PE cycles/row.  Accuracy is recovered
    with a hi+lo split of BOTH operands at a shared power-of-2 scale
    (x = xh + xl, w = wh + wl, all e4m3 at the same scale), computing
    xh*wh + xl*wh + xh*wl in one PSUM accumulation group.  The dropped
    xl*wl term is ~0.1% relative -> ~0.2-0.3% on the final output.
    Per-tensor scales are free: BN is scale-invariant (eps rescaled).
  * BN batch stats cross-core exchange via small AllGather collectives
    (cheaper than AllReduce) + local sum; stats for c1 and csc are overlapped
    with compute, only the c2-stats gather sits on the tail.
"""

import numpy as np
import ml_dtypes

NCORES = 8
B = 16
B_LOC = B // NCORES          # 2 images per core
CIN, COUT = 1024, 512
NCI, NCO = CIN // 128, COUT // 128   # 8, 4 partition tiles
NPAIR = NCI // 2              # 4 cin-tile pairs (DoubleRow k-groups)
H = 16                        # input spatial
OH = 32                       # output spatial
CNT = float(B * OH * OH)      # BN element count per channel = 16384
PHASES = [(0, 0), (0, 1), (1, 0), (1, 1)]

SX = 32.0                     # x quantization scale
SW = 1024.0                   # conv5 weight scale (w1, wsc)
SA = 32.0                     # out1 activation scale
SW2 = 1024.0                  # conv2 weight scale
ALPHA = SX * SW               # = SA * SW2: scale of every conv output
EPS_S = 1e-5 * ALPHA * ALPHA  # BN eps in the scaled-variance domain

F8NP = ml_dtypes.float8_e4m3

_CACHE = {}


def _taps(p):
    """Taps of a parity phase along one dim: list of (input shift, 5-tap kernel idx)."""
    if p == 0:
        return [(-1, 0), (0, 2), (1, 4)]
    return [(0, 1), (1, 3)]


def _w5_groups():
    """Weight-block groups for the phase-decomposed 5x5 conv in consumption
    order: one group per (phase, cin-pair, kernel-row) holding len(kws) blocks
    of [128 k, 2 member, COUT]."""
    groups = []
    for (p, q) in PHASES:
        for j in range(NPAIR):
            for (ah, kh) in _taps(p):
                groups.append((p, q, j, ah, kh, _taps(q)))
    return groups


def _phase_view(ap2048, p, q):
    """[128, 2048] tile viewed as [128, b, i, j] at output positions (2i+p, 2j+q)."""
    v = ap2048.rearrange("c (b i p2 j q2) -> c b i p2 j q2", b=2, i=16, p2=2, j=16, q2=2)
    return v[:, :, :, p, :, q]


def _build_nc():
    import concourse.mybir as mybir
    import concourse.tile as tile
    from concourse import bacc

    f32 = mybir.dt.float32
    f8 = mybir.dt.float8e4
    ALU = mybir.AluOpType
    AFT = mybir.ActivationFunctionType
    DR = mybir.MatmulPerfMode.DoubleRow

    nc = bacc.Bacc("TRN2", target_bir_lowering=False, debug=False)

    # ---- kernel I/O ----
    xh_d = nc.dram_tensor("xh", [NPAIR, 128, 2, B_LOC, 18, 18], f8, kind="ExternalInput").ap()
    xl_d = nc.dram_tensor("xl", [NPAIR, 128, 2, B_LOC, 18, 18], f8, kind="ExternalInput").ap()
    w1h_d = nc.dram_tensor("w1h", [100, 128, 2, COUT], f8, kind="ExternalInput").ap()
    w1l_d = nc.dram_tensor("w1l", [100, 128, 2, COUT], f8, kind="ExternalInput").ap()
    wsch_d = nc.dram_tensor("wsch", [100, 128, 2, COUT], f8, kind="ExternalInput").ap()
    wscl_d = nc.dram_tensor("wscl", [100, 128, 2, COUT], f8, kind="ExternalInput").ap()
    w2h_d = nc.dram_tensor("w2h", [2, 128, 9, 2, COUT], f8, kind="ExternalInput").ap()
    w2l_d = nc.dram_tensor("w2l", [2, 128, 9, 2, COUT], f8, kind="ExternalInput").ap()
    gb_d = nc.dram_tensor("gb", [128, 6, 4], f32, kind="ExternalInput").ap()
    zp_d = nc.dram_tensor("zp", [2, B_LOC, 34, 34], f8, kind="ExternalInput").ap()
    out_d = nc.dram_tensor("out", [B_LOC, COUT, OH, OH], f32, kind="ExternalOutput").ap()

    with tile.TileContext(nc) as tc:
        # collective buffers (internal DRAM)
        _frees = []

        def dram(shape, shared=False, name=""):
            t, _f = tc.tile(shape, f32, space="DRAM",
                            addr_space="Shared" if shared else None, name=name)
            _frees.append(_f)
            return t

        # stats layouts are [s(sum,sq), c(128), co] so DMAs keep co contiguous
        ag1_in = dram([2, 128, 4], name="ag1_in")
        ag1_out = dram([NCORES, 2, 128, 4], shared=True, name="ag1_out")
        agsc_in = dram([2, 128, 4], name="agsc_in")
        agsc_out = dram([NCORES, 2, 128, 4], shared=True, name="agsc_out")
        ag2a_in = dram([2, 128, 2], name="ag2a_in")
        ag2a_out = dram([NCORES, 2, 128, 2], shared=True, name="ag2a_out")
        ag2b_in = dram([2, 128, 1], name="ag2b_in")
        ag2b_out = dram([NCORES, 2, 128, 1], shared=True, name="ag2b_out")
        ag2c_in = dram([2, 128, 1], name="ag2c_in")
        ag2c_out = dram([NCORES, 2, 128, 1], shared=True, name="ag2c_out")

        with tc.tile_pool(name="xp", bufs=1) as xp_pool, \
             tc.tile_pool(name="acts", bufs=1) as acts, \
             tc.tile_pool(name="op1", bufs=1) as op1_pool, \
             tc.tile_pool(name="w2p", bufs=1) as w2p, \
             tc.tile_pool(name="wts", bufs=8) as wts, \
             tc.tile_pool(name="tsc", bufs=2) as tsc_pool, \
             tc.tile_pool(name="scr", bufs=1) as scr_pool, \
             tc.tile_pool(name="small", bufs=1) as small, \
             tc.tile_pool(name="ps", bufs=8, space="PSUM") as ps:

            # ---- persistent SBUF tensors ----
            XH = [xp_pool.tile([128, 2, B_LOC, 18, 18], f8, name=f"xh{j}", tag=f"xh{j}")
                  for j in range(NPAIR)]
            XL = [xp_pool.tile([128, 2, B_LOC, 18, 18], f8, name=f"xl{j}", tag=f"xl{j}")
                  for j in range(NPAIR)]
            C1 = [acts.tile([128, 2048], f32, name=f"c1_{i}", tag=f"c1_{i}") for i in range(NCO)]
            CSC = [acts.tile([128, 2048], f32, name=f"csc_{i}", tag=f"csc_{i}") for i in range(NCO)]
            C2 = C1           # conv1 results are dead once BN1 is applied
            FIN = CSC         # csc results are dead once the final fuse read them
            O1H = [op1_pool.tile([128, 2, B_LOC, 34, 34], f8, name=f"o1h{j}", tag=f"o1h{j}")
                   for j in range(2)]
            O1L = [op1_pool.tile([128, 2, B_LOC, 34, 34], f8, name=f"o1l{j}", tag=f"o1l{j}")
                   for j in range(2)]
            W2H = [w2p.tile([128, 9, 2, COUT], f8, name=f"w2h{cp}", tag=f"w2h{cp}")
                   for cp in range(2)]
            W2L = [w2p.tile([128, 9, 2, COUT], f8, name=f"w2l{cp}", tag=f"w2l{cp}")
                   for cp in range(2)]

            # stat columns: sums/sumsqs per (tensor, co, phase-or-quarter)
            sums1 = small.tile([128, 16], f32, name="sums1")
            sq1 = small.tile([128, 16], f32, name="sq1")
            sums2 = small.tile([128, 16], f32, name="sums2")
            sq2 = small.tile([128, 16], f32, name="sq2")
            sumssc = small.tile([128, 16], f32, name="sumssc")
            sqsc = small.tile([128, 16], f32, name="sqsc")
            pack1 = small.tile([128, 2, 4], f32, name="pack1")
            packsc = small.tile([128, 2, 4], f32, name="packsc")
            pack2a = small.tile([128, 2, 2], f32, name="pack2a")
            pack2b = small.tile([128, 2, 1], f32, name="pack2b")
            pack2c = small.tile([128, 2, 1], f32, name="pack2c")
            st1g = small.tile([128, NCORES, 2, 4], f32, name="st1g")
            stscg = small.tile([128, NCORES, 2, 4], f32, name="stscg")
            st2ga = small.tile([128, NCORES, 2, 2], f32, name="st2ga")
            st2gb = small.tile([128, NCORES, 2], f32, name="st2gb")
            st2gc = small.tile([128, NCORES, 2], f32, name="st2gc")
            st1 = small.tile([128, 2, 4], f32, name="st1")
            st2 = small.tile([128, 2, 2, 4], f32, name="st2")   # [c, grp(c2,sc), (sum,sq), co]
            gbv = small.tile([128, 6, 4], f32, name="gbv")      # rows: SA*g1, SA*b1, g2, b2, gsc, bsc
            scale1 = small.tile([128, 4], f32, name="scale1")
            shift1 = small.tile([128, 4], f32, name="shift1")
            tmpa = small.tile([128, 4], f32, name="tmpa")
            tmpb = small.tile([128, 4], f32, name="tmpb")
            epsc = small.tile([128, 1], f32, name="epsc")
            # two independent sets of BN2/BNsc coefficient tiles (pass a: cos
            # 0-1 fused early; pass b: cos 2-3 fused on the tail)
            coefs = {}
            for ph in ("a", "b", "c"):
                coefs[ph] = {
                    "stm": small.tile([128, 2, 2, 4], f32, name=f"stm_{ph}"),
                    "m2": small.tile([128, 2, 4], f32, name=f"m2_{ph}"),
                    "var": small.tile([128, 2, 4], f32, name=f"var_{ph}"),
                    "inv": small.tile([128, 2, 4], f32, name=f"inv_{ph}"),
                    "scaleb": small.tile([128, 2, 4], f32, name=f"scaleb_{ph}"),
                    "shiftb2": small.tile([128, 2, 4], f32, name=f"shiftb2_{ph}"),
                    "shiftB": small.tile([128, 4], f32, name=f"shiftB_{ph}"),
                    "rmix": small.tile([128, 4], f32, name=f"rmix_{ph}"),
                    "tmp": small.tile([128, 4], f32, name=f"tmp_{ph}"),
                }

            # ---- input DMAs (x first: the PE's first dependency) ----
            x_emitted = [True] + [False] * (NPAIR - 1)

            def emit_x(j):
                if not x_emitted[j]:
                    x_emitted[j] = True
                    nc.sync.dma_start(XH[j][:], xh_d[j])
                    nc.sync.dma_start(XL[j][:], xl_d[j])

            nc.sync.dma_start(XH[0][:], xh_d[0])
            nc.vector.memset(epsc[:], EPS_S)
            nc.vector.memset(st2[:], 1.0)

            def gsum(dst, gtile, w):
                """dst[c, 2, W] = sum over cores of gtile[c, 8, 2, W]."""
                nc.vector.tensor_reduce(
                    dst, gtile[:].rearrange("c g s co -> c s co g"),
                    axis=mybir.AxisListType.X, op=ALU.add)

            # ---- helper: one 5x5-phase-decomposed conv (conv1 / convsc) ----
            def conv5(wh_d, wl_d, dst, sums, sqs, wtag, prefetch_x=False):
                gofs = 0
                for iph, (p, q) in enumerate(PHASES):
                    pps = [ps.tile([128, B_LOC, 256], f32, name=f"{wtag}ps{iph}_{co}", tag="psb")
                           for co in range(NCO)]
                    kws = _taps(q)
                    L = len(kws)
                    total = NPAIR * len(_taps(p)) * L * 3 * B_LOC
                    cnt = [0] * NCO
                    for j in range(NPAIR):
                        if prefetch_x and iph == 0 and j + 1 < NPAIR:
                            emit_x(j + 1)
                        for (ah, kh) in _taps(p):
                            wh = wts.tile([128, 3, 2, COUT], f8, name=f"{wtag}wh", tag="w5")
                            wl = wts.tile([128, 3, 2, COUT], f8, name=f"{wtag}wl", tag="w5")
                            if gofs == 0 and prefetch_x:
                                # split the very first group so the first
                                # matmul's weights land as early as possible;
                                # xl0 is only needed 8 matmuls in, so it rides
                                # behind the first hi-path blocks
                                nc.sync.dma_start(
                                    wh[:, 0:1], wh_d[0:1].rearrange("l k m o -> k l m o"))
                                nc.sync.dma_start(
                                    wl[:, 0:1], wl_d[0:1].rearrange("l k m o -> k l m o"))
                                nc.sync.dma_start(XL[0][:], xl_d[0])
                                nc.sync.dma_start(
                                    wh[:, 1:L], wh_d[1:L].rearrange("l k m o -> k l m o"))
                                nc.sync.dma_start(
                                    wl[:, 1:L], wl_d[1:L].rearrange("l k m o -> k l m o"))
                            else:
                                nc.sync.dma_start(
                                    wh[:, :L], wh_d[gofs:gofs + L].rearrange("l k m o -> k l m o"))
                                nc.sync.dma_start(
                                    wl[:, :L], wl_d[gofs:gofs + L].rearrange("l k m o -> k l m o"))
                            gofs += L
                            for kwi, (aw, kw) in enumerate(kws):
                                for (xt, wt) in ((XH, wh), (XL, wh), (XH, wl)):
                                    for b in range(B_LOC):
                                        rhs = xt[j][:, :, b, 1 + ah:17 + ah, 1 + aw:17 + aw]
                                        for co in range(NCO):
                                            nc.tensor.matmul(
                                                pps[co][:, b],
                                                wt[:, kwi, :, co * 128:(co + 1) * 128],
                                                rhs,
                                                start=(cnt[co] == 0),
                                                stop=(cnt[co] == total - 1),
                                                perf_mode=DR)
                                            cnt[co] += 1
                    for co in range(NCO):
                        icol = co * 4 + iph
                        nc.vector.tensor_scalar(
                            dst[co][:, iph * 512:(iph + 1) * 512],
                            pps[co][:].rearrange("c b n -> c (b n)"),
                            0.0, 0.0, op0=ALU.add, op1=ALU.add,
                            accum_out=sums[:, icol:icol + 1])
                        scr = scr_pool.tile([128, 512], f32, name=f"{wtag}scr", tag="scr")
                        nc.scalar.activation(
                            scr[:], pps[co][:].rearrange("c b n -> c (b n)"), AFT.Square,
                            accum_out=sqs[:, icol:icol + 1])

            # ================= conv1 =================
            conv5(w1h_d, w1l_d, C1, sums1, sq1, "c1", prefetch_x=True)

            # aux DMAs (needed later; emitted after the conv1 weight stream so
            # they don't delay it on the DMA engines)
            nc.sync.dma_start(gbv[:], gb_d)
            for cp in range(2):
                nc.sync.dma_start(W2H[cp][:], w2h_d[cp])
                nc.sync.dma_start(W2L[cp][:], w2l_d[cp])
                nc.sync.dma_start(O1H[cp][:], zp_d.unsqueeze(0).partition_broadcast(128))
                nc.sync.dma_start(O1L[cp][:], zp_d.unsqueeze(0).partition_broadcast(128))

            # ---- c1 stats -> AllGather #1 (overlaps with convsc compute) ----
            nc.vector.tensor_reduce(
                pack1[:, 0, :], sums1[:].rearrange("c (co ph) -> c co ph", ph=4),
                axis=mybir.AxisListType.X, op=ALU.add)
            nc.vector.tensor_reduce(
                pack1[:, 1, :], sq1[:].rearrange("c (co ph) -> c co ph", ph=4),
                axis=mybir.AxisListType.X, op=ALU.add)
            nc.sync.dma_start(ag1_in[:].rearrange("s c co -> c s co"), pack1[:])
            nc.gpsimd.collective_compute(
                "AllGather", ALU.bypass,
                replica_groups=[list(range(NCORES))],
                ins=[ag1_in.opt()], outs=[ag1_out.opt()])

            # ================= convsc (independent of BN1) =================
            conv5(wsch_d, wscl_d, CSC, sumssc, sqsc, "sc")

            # ---- csc stats -> AllGather (overlaps with conv2) ----
            nc.vector.tensor_reduce(
                packsc[:, 0, :], sumssc[:].rearrange("c (co ph) -> c co ph", ph=4),
                axis=mybir.AxisListType.X, op=ALU.add)
            nc.vector.tensor_reduce(
                packsc[:, 1, :], sqsc[:].rearrange("c (co ph) -> c co ph", ph=4),
                axis=mybir.AxisListType.X, op=ALU.add)
            nc.sync.dma_start(agsc_in[:].rearrange("s c co -> c s co"), packsc[:])
            nc.gpsimd.collective_compute(
                "AllGather", ALU.bypass,
                replica_groups=[list(range(NCORES))],
                ins=[agsc_in.opt()], outs=[agsc_out.opt()])

            # ---- BN1 scale/shift from global stats (incl. SA requant scale) ----
            nc.sync.dma_start(st1g[:], ag1_out[:].rearrange("g s c co -> c g s co"))
            gsum(st1[:], st1g, 4)
            nc.vector.tensor_scalar_mul(st1[:], st1[:], 1.0 / CNT)
            m1 = st1[:, 0, :]
            nc.vector.tensor_tensor(tmpa[:], m1, m1, op=ALU.mult)
            nc.vector.tensor_tensor(tmpb[:], st1[:, 1, :], tmpa[:], op=ALU.subtract)
            nc.scalar.activation(tmpb[:], tmpb[:], AFT.Sqrt, bias=epsc[:])
            nc.vector.reciprocal(tmpa[:], tmpb[:])
            nc.vector.tensor_tensor(scale1[:], gbv[:, 0, :], tmpa[:], op=ALU.mult)
            nc.vector.tensor_tensor(tmpa[:], m1, scale1[:], op=ALU.mult)
            nc.vector.tensor_tensor(shift1[:], gbv[:, 1, :], tmpa[:], op=ALU.subtract)

            # ---- BN1 apply + ReLU + fp8 hi/lo requant -> padded conv2 input ----
            for co in range(NCO):
                cp, m = co // 2, co % 2
                for iph, (p, q) in enumerate(PHASES):
                    src = C1[co][:, iph * 512:(iph + 1) * 512]
                    T = tsc_pool.tile([128, 512], f32, name="bn1T", tag="bn1T")
                    nc.scalar.activation(T[:], src, AFT.Relu,
                                         bias=shift1[:, co:co + 1], scale=scale1[:, co:co + 1])
                    Tv = T[:].rearrange("c (b h w) -> c b h w", b=B_LOC, h=16)
                    hdst = O1H[cp][:, m, :, 1:33, 1:33] \
                        .rearrange("c b (i p2) (jj q2) -> c b i p2 jj q2", p2=2, q2=2)[:, :, :, p, :, q]
                    ldst = O1L[cp][:, m, :, 1:33, 1:33] \
                        .rearrange("c b (i p2) (jj q2) -> c b i p2 jj q2", p2=2, q2=2)[:, :, :, p, :, q]
                    nc.scalar.activation(hdst, Tv, AFT.Copy)
                    nc.vector.tensor_tensor(ldst, Tv, hdst, op=ALU.subtract)

            # ================= conv2 (3x3, pad 1, on O1 fp8 hi/lo) =================
            # quarter-outer loop: each quarter's PSUM group completes early so
            # its copy-out overlaps the next quarter's matmuls.  For the LAST
            # co the stats come from a cheap tensor_reduce instead of the
            # (strided, slow) copy's accumulator, so the tail collective isn't
            # gated on the copy.
            def conv2_co(co, split_stats=False):
                """split_stats: stats come from a cheap reduce + ACT square so
                the tail collective isn't gated on the slow strided C2 copies;
                those copies are returned for deferred (lower-priority)
                emission — they only gate the post-collective fuse."""
                total = 2 * 9 * 3 * B_LOC
                deferred = []
                for qq in range(4):
                    pp = ps.tile([128, B_LOC, 256], f32, name=f"c2ps{co}_{qq}", tag="psb")
                    cnt = 0
                    for cp in range(2):
                        for dh in (-1, 0, 1):
                            for dw in (-1, 0, 1):
                                t = (dh + 1) * 3 + (dw + 1)
                                for (xt, wt) in ((O1H, W2H), (O1L, W2H), (O1H, W2L)):
                                    for b in range(B_LOC):
                                        rhs = xt[cp][:, :, b,
                                                     1 + 8 * qq + dh:9 + 8 * qq + dh,
                                                     1 + dw:33 + dw]
                                        nc.tensor.matmul(
                                            pp[:, b],
                                            wt[cp][:, t, :, co * 128:(co + 1) * 128],
                                            rhs,
                                            start=(cnt == 0),
                                            stop=(cnt == total - 1),
                                            perf_mode=DR)
                                        cnt += 1
                    icol = co * 4 + qq
                    dstv = C2[co][:].rearrange("c (b h w) -> c b h w", b=B_LOC, h=32)[:, :, 8 * qq:8 * qq + 8, :]
                    scr = scr_pool.tile([128, 512], f32, name="c2scr", tag="scr")
                    if split_stats:
                        pf = pp[:].rearrange("c b n -> c (b n)")
                        nc.vector.tensor_reduce(
                            sums2[:, icol:icol + 1], pf,
                            axis=mybir.AxisListType.X, op=ALU.add)
                        nc.scalar.activation(
                            scr[:], pf, AFT.Square,
                            accum_out=sq2[:, icol:icol + 1])
                        deferred.append((qq, pp, dstv))
                    else:
                        nc.vector.tensor_scalar(
                            dstv, pp[:].rearrange("c b (h w) -> c b h w", h=8),
                            0.0, 0.0, op0=ALU.add, op1=ALU.add,
                            accum_out=sums2[:, icol:icol + 1])
                        nc.scalar.activation(
                            scr[:], pp[:].rearrange("c b n -> c (b n)"), AFT.Square,
                            accum_out=sq2[:, icol:icol + 1])
                return deferred

            def conv2_copies(deferred):
                for (qq, pp, dstv) in deferred:
                    nc.vector.tensor_scalar(
                        dstv, pp[:].rearrange("c b (h w) -> c b h w", h=8),
                        0.0, 0.0, op0=ALU.add, op1=ALU.add)

            def pack_c2(pk, ag_in, ag_out, lo, hi):
                nc.vector.tensor_reduce(
                    pk[:, 0, :], sums2[:, lo * 4:hi * 4].rearrange("c (co x) -> c co x", x=4),
                    axis=mybir.AxisListType.X, op=ALU.add)
                nc.vector.tensor_reduce(
                    pk[:, 1, :], sq2[:, lo * 4:hi * 4].rearrange("c (co x) -> c co x", x=4),
                    axis=mybir.AxisListType.X, op=ALU.add)
                nc.sync.dma_start(ag_in[:].rearrange("s c co -> c s co"), pk[:])
                nc.gpsimd.collective_compute(
                    "AllGather", ALU.bypass,
                    replica_groups=[list(range(NCORES))],
                    ins=[ag_in.opt()], outs=[ag_out.opt()])

            def bn2_coefs(cf):
                """BN2/BNsc scale+shift from st2 into the given coef tile set.
                final = relu(s2*c2 + t2 + ssc*csc + tsc)
                      = relu( s2 * (c2 + (ssc/s2)*csc) + (t2 + tsc) )"""
                stm = cf["stm"]
                nc.vector.tensor_scalar_mul(stm[:], st2[:], 1.0 / CNT)
                means = stm[:, :, 0, :]    # [c, 2, 4]
                e2s = stm[:, :, 1, :]
                gpair = gbv[:, 2:, :].rearrange("c (g s) co -> c g s co", s=2)
                nc.vector.tensor_tensor(cf["m2"][:], means, means, op=ALU.mult)
                nc.vector.tensor_tensor(cf["var"][:], e2s, cf["m2"][:], op=ALU.subtract)
                nc.scalar.activation(cf["var"][:], cf["var"][:], AFT.Sqrt, bias=epsc[:])
                nc.vector.reciprocal(cf["inv"][:], cf["var"][:])
                nc.vector.tensor_tensor(cf["scaleb"][:], gpair[:, :, 0, :], cf["inv"][:], op=ALU.mult)
                nc.vector.tensor_tensor(cf["m2"][:], means, cf["scaleb"][:], op=ALU.mult)
                nc.vector.tensor_tensor(cf["shiftb2"][:], gpair[:, :, 1, :], cf["m2"][:], op=ALU.subtract)
                nc.vector.tensor_tensor(cf["shiftB"][:], cf["shiftb2"][:, 0, :],
                                        cf["shiftb2"][:, 1, :], op=ALU.add)
                nc.vector.reciprocal(cf["tmp"][:], cf["scaleb"][:, 0, :])
                nc.vector.tensor_tensor(cf["rmix"][:], cf["scaleb"][:, 1, :], cf["tmp"][:], op=ALU.mult)

            def fuse_co(co, cf):
                """c2 += rmix*csc ; out = relu(scale2*c2 + shiftB) ; DMA out."""
                for iph, (p, q) in enumerate(PHASES):
                    nc.vector.scalar_tensor_tensor(
                        _phase_view(C2[co][:], p, q),
                        CSC[co][:, iph * 512:(iph + 1) * 512]
                        .rearrange("c (b h w) -> c b h w", b=B_LOC, h=16),
                        cf["rmix"][:, co:co + 1],
                        _phase_view(C2[co][:], p, q),
                        op0=ALU.mult, op1=ALU.add)
                fin = FIN[co]
                for b in range(B_LOC):
                    nc.scalar.activation(fin[:, b * 1024:(b + 1) * 1024],
                                         C2[co][:, b * 1024:(b + 1) * 1024], AFT.Relu,
                                         bias=cf["shiftB"][:, co:co + 1],
                                         scale=cf["scaleb"][:, 0, co:co + 1])
                    nc.scalar.dma_start(
                        out_d[b, co * 128:(co + 1) * 128].rearrange("c h w -> c (h w)"),
                        fin[:, b * 1024:(b + 1) * 1024])

            conv2_co(0)
            conv2_co(1)
            # stats for cos 0-1 -> AllGather (hidden under cos 2-3 conv2);
            # their BN coefs + fuse + output DMA also overlap cos 2-3 compute.
            pack_c2(pack2a, ag2a_in, ag2a_out, 0, 2)
            conv2_co(2)
            pack_c2(pack2b, ag2b_in, ag2b_out, 2, 3)
            d3 = conv2_co(3, split_stats=True)
            pack_c2(pack2c, ag2c_in, ag2c_out, 3, 4)
            conv2_copies(d3)

            # gather-backs + local sums (csc stats gathered during conv2)
            nc.sync.dma_start(stscg[:], agsc_out[:].rearrange("g s c co -> c g s co"))
            gsum(st2[:, 1], stscg, 4)
            nc.sync.dma_start(st2ga[:], ag2a_out[:].rearrange("g s c co -> c g s co"))
            gsum(st2[:, 0, :, 0:2], st2ga, 2)
            bn2_coefs(coefs["a"])
            fuse_co(0, coefs["a"])
            fuse_co(1, coefs["a"])

            nc.sync.dma_start(st2gb[:], ag2b_out[:].rearrange("g s c co -> c g (s co)"))
            nc.vector.tensor_reduce(st2[:, 0, :, 2], st2gb[:].rearrange("c g s -> c s g"), axis=mybir.AxisListType.X, op=ALU.add)
            bn2_coefs(coefs["b"])
            fuse_co(2, coefs["b"])

            nc.sync.dma_start(st2gc[:], ag2c_out[:].rearrange("g s c co -> c g (s co)"))
            nc.vector.tensor_reduce(st2[:, 0, :, 3], st2gc[:].rearrange("c g s -> c s g"), axis=mybir.AxisListType.X, op=ALU.add)
            bn2_coefs(coefs["c"])
            fuse_co(3, coefs["c"])

            for _f in _frees:
                _f()

    nc.compile()
    return nc


def _get_nc():
    if "nc" not in _CACHE:
        _CACHE["nc"] = _build_nc()
    return _CACHE["nc"]


def _q8pair(a):
    """Split fp32 array into e4m3 hi + lo at unit scale (caller pre-scales)."""
    hi = a.astype(F8NP)
    lo = (a - hi.astype(np.float32)).astype(F8NP)
    return hi, lo


def _regroup_w5(wt_full):
    """[5,5,CIN,COUT] (pre-scaled fp32) -> [100, 128, 2, COUT] blocks in
    kernel consumption order."""
    blocks = np.empty((100, 128, 2, COUT), dtype=np.float32)
    g = 0
    for (p, q, j, ah, kh, kws) in _w5_groups():
        for (aw, kw) in kws:
            blk = wt_full[kh, kw, j * 256:(j + 1) * 256, :]          # [256, COUT]
            blocks[g] = blk.reshape(2, 128, COUT).transpose(1, 0, 2)  # [k, m, o]
            g += 1
    assert g == 100
    return blocks


def _prep_core_x(xpad_s):
    """Scaled padded x for one core [B_LOC, CIN, 18, 18] ->
    [NPAIR, 128, 2, B_LOC, 18, 18] (pair, k, member, img, h, w)."""
    a = xpad_s.reshape(B_LOC, NPAIR, 2, 128, 18, 18)
    return np.ascontiguousarray(a.transpose(1, 3, 2, 0, 4, 5))


def _prep_inputs(x, w1, w2, wsc, g1, b1, g2, b2, gsc, bsc):
    xpad = np.zeros((B, CIN, 18, 18), dtype=np.float32)
    xpad[:, :, 1:17, 1:17] = x
    xpad *= SX

    w1g = _regroup_w5(np.ascontiguousarray(w1.transpose(2, 3, 1, 0)) * SW)
    wscg = _regroup_w5(np.ascontiguousarray(wsc.transpose(2, 3, 1, 0)) * SW)
    w1h, w1l = _q8pair(w1g)
    wsch, wscl = _q8pair(wscg)

    # w2: [COUT, COUT, 3, 3] -> [cp, k, t, m, o]
    wt2 = np.ascontiguousarray(w2.transpose(2, 3, 1, 0)).reshape(9, COUT, COUT) * SW2
    w2g = np.ascontiguousarray(
        wt2.reshape(9, 2, 2, 128, COUT).transpose(1, 3, 0, 2, 4))  # [cp, k, t, m, o]
    w2h, w2l = _q8pair(w2g)

    gb = np.stack([g1 * SA, b1 * SA, g2, b2, gsc, bsc]).astype(np.float32)   # [6, 512]
    gbt = np.ascontiguousarray(gb.reshape(6, 4, 128).transpose(2, 0, 1))  # [128, 6, 4]
    return xpad, (w1h, w1l), (wsch, wscl), (w2h, w2l), gbt


def kernel(x, w1, g1, b1, w2, g2, b2, wsc, gsc, bsc, _trace=False, **_kw):
    from concourse.bass_utils import run_bass_kernel_spmd

    x = np.asarray(x, dtype=np.float32)
    xpad, (w1h, w1l), (wsch, wscl), (w2h, w2l), gbt = _prep_inputs(
        np.asarray(x), np.asarray(w1), np.asarray(w2), np.asarray(wsc),
        np.asarray(g1), np.asarray(b1), np.asarray(g2), np.asarray(b2),
        np.asarray(gsc), np.asarray(bsc))

    nc = _get_nc()
    zp = np.zeros((2, B_LOC, 34, 34), dtype=F8NP)
    in_maps = []
    for core in range(NCORES):
        xs = _prep_core_x(xpad[core * B_LOC:(core + 1) * B_LOC])
        xh, xl = _q8pair(xs)
        in_maps.append({
            "xh": xh, "xl": xl,
            "w1h": w1h, "w1l": w1l, "wsch": wsch, "wscl": wscl,
            "w2h": w2h, "w2l": w2l, "gb": gbt, "zp": zp,
        })
    try:
        res = run_bass_kernel_spmd(nc, in_maps, list(range(NCORES)), trace=_trace)
    except ModuleNotFoundError:
        # NTFF profile hook unavailable under this axon client
        res = run_bass_kernel_spmd(nc, in_maps, list(range(NCORES)), trace=False)
    out = np.concatenate([res.results[i]["out"] for i in range(NCORES)], axis=0)
    if _trace:
        _CACHE["last_result"] = res
    return out
